# revision 2
# baseline (speedup 1.0000x reference)
"""Grok1-style MoE (T=2048, H=1024, E=8, I=2048, top-2) on 8 Trainium2 cores.

Strategy (expert-parallel, per the sharding hint):
  - Host: compute the tiny router (x @ gate_w, tanh softcap, top-2, softmax)
    and dispatch tokens by expert assignment (the "all-to-all dispatch" step:
    with full inputs on the host, dispatch = gather per expert), packing the
    per-core shards in the device-friendly tiled layout.
  - Device (SPMD, 1 expert per core), all-bf16 matmuls (enables the PE's
    fast-weight-load path and halves HBM traffic; fp32 PSUM accumulate):
      Phase 1:  gT/uT[i, m] = wg_e^T x_e^T / wu_e^T x_e^T  per 128-i-tile
                act[i, m]   = gelu_tanh(gT) * uT            (stored bf16)
      Phase 2:  yT[h, m]    = sum_i wd_e[i, h] * act[i, m]  (exact-M moving
                dim -- no 128-token quantization of the output pass)
  - Host: combine = out[ids] += prob * yT.T per expert (prob scaling is
    linear in the expert output, so it folds into the host-side combine).
"""

import numpy as np
import ml_dtypes

import concourse.mybir as mybir
import concourse.tile as tile
from concourse import bacc
from concourse.bass_utils import run_bass_kernel_spmd

T, H, E, I_DIM, TOPK = 2048, 1024, 8, 2048, 2
SOFTCAP = 30.0
P = 128
N_CORES = 8
KH = H // P      # 8 contraction tiles (phase 1)
NI = I_DIM // P  # 16 i tiles
HT = H // P      # 8 h tiles (phase 2 output partitions)

BF16 = ml_dtypes.bfloat16

_compiled = {}
LAST_RESULTS = None


def _build(M_PAD):
    c0 = M_PAD // 2
    chunks = ((0, c0), (c0, M_PAD - c0))
    f32 = mybir.dt.float32
    bf16 = mybir.dt.bfloat16

    nc = bacc.Bacc("TRN2", target_bir_lowering=False, num_devices=N_CORES)
    # Host-packed layouts (all DMAs contiguous per partition):
    #   xt  [KH, P, M_PAD]   : xt[k, p, m] = x_e[m, k*P+p]
    #   wg  [NI, P, KH*P]    : wg[it, p, k*P+i] = wg_e[k*P+p, it*P+i]
    #   wu  same as wg
    #   wd  [I, H]           : natural layout (row-tile slices are contiguous)
    #   y   [HT, P, M_PAD]   : y[ht, p, m] = yT_e[ht*P+p, m]
    xt = nc.dram_tensor("xt", [KH, P, M_PAD], bf16, kind="ExternalInput")
    wg = nc.dram_tensor("wg", [NI, P, KH * P], bf16, kind="ExternalInput")
    wu = nc.dram_tensor("wu", [NI, P, KH * P], bf16, kind="ExternalInput")
    wd = nc.dram_tensor("wd", [I_DIM, H], bf16, kind="ExternalInput")
    y = nc.dram_tensor("y", [HT, P, M_PAD], f32, kind="ExternalOutput")

    with tile.TileContext(nc) as tc:
        with (
            tc.tile_pool(name="persist", bufs=1) as persist,
            tc.tile_pool(name="wtiles", bufs=4) as wtiles,
            tc.tile_pool(name="youts", bufs=2) as youts,
            tc.tile_pool(name="psA", bufs=1, space="PSUM") as psA,
            tc.tile_pool(name="psB", bufs=2, space="PSUM") as psB,
        ):
            xt_sb = persist.tile([P, KH, M_PAD], bf16)
            wd_sb = persist.tile([P, NI, H], bf16)
            acts = persist.tile([P, NI, M_PAD], bf16)

            def w_src(w, it):
                return w.ap()[it].rearrange("p (k i) -> p k i", i=P)

            wg_sbs, wu_sbs = {}, {}

            def load_w(it):
                wg_sbs[it] = wtiles.tile([P, KH, P], bf16, tag="wg", name=f"wg{it}")
                nc.sync.dma_start(wg_sbs[it][:], w_src(wg, it))
                wu_sbs[it] = wtiles.tile([P, KH, P], bf16, tag="wu", name=f"wu{it}")
                nc.scalar.dma_start(wu_sbs[it][:], w_src(wu, it))

            # Startup: first weight tiles, then xt k-slices split across both
            # HWDGE rings, so the first matmul unblocks on small transfers.
            load_w(0)
            for k in range(KH):
                eng = nc.sync if k % 2 == 0 else nc.scalar
                eng.dma_start(xt_sb[:, k], xt.ap()[k])
            load_w(1)

            gelu = mybir.ActivationFunctionType.Gelu_apprx_tanh

            # Phase 1: per i-tile: gT/uT = wg^T xT / wu^T xT; act = gelu(g)*u
            for it in range(NI):
                pf = it + 2
                if pf < NI and pf not in wg_sbs:
                    load_w(pf)
                # wd tile loads are spread over the phase-1 steady state
                # (consumed only in phase 2).
                if 1 <= it <= 8:
                    w0 = 2 * (it - 1)
                    nc.sync.dma_start(
                        wd_sb[:, w0], wd.ap()[w0 * P:(w0 + 1) * P, :]
                    )
                    nc.scalar.dma_start(
                        wd_sb[:, w0 + 1], wd.ap()[(w0 + 1) * P:(w0 + 2) * P, :]
                    )

                wg_sb, wu_sb = wg_sbs.pop(it), wu_sbs.pop(it)
                g_ps = [
                    psA.tile([P, ln], f32, tag=f"g{j}", name=f"g{j}_{it}")
                    for j, (m0, ln) in enumerate(chunks)
                ]
                for k in range(KH):
                    for j, (m0, ln) in enumerate(chunks):
                        nc.tensor.matmul(
                            g_ps[j][:],
                            wg_sb[:, k],
                            xt_sb[:, k, m0:m0 + ln],
                            start=(k == 0),
                            stop=(k == KH - 1),
                        )
                u_ps = [
                    psA.tile([P, ln], f32, tag=f"u{j}", name=f"u{j}_{it}")
                    for j, (m0, ln) in enumerate(chunks)
                ]
                for k in range(KH):
                    for j, (m0, ln) in enumerate(chunks):
                        nc.tensor.matmul(
                            u_ps[j][:],
                            wu_sb[:, k],
                            xt_sb[:, k, m0:m0 + ln],
                            start=(k == 0),
                            stop=(k == KH - 1),
                        )
                for j, (m0, ln) in enumerate(chunks):
                    nc.scalar.activation(acts[:, it, m0:m0 + ln], g_ps[j][:], gelu)
                    nc.vector.tensor_mul(
                        acts[:, it, m0:m0 + ln], acts[:, it, m0:m0 + ln], u_ps[j][:]
                    )

            # Phase 2: yT[h, m] = sum_i wd[i, h] * act[i, m]
            for ht in range(HT):
                d_ps = [
                    psB.tile([P, ln], f32, tag=f"d{j}", name=f"d{j}_{ht}")
                    for j, (m0, ln) in enumerate(chunks)
                ]
                for it in range(NI):
                    wslice = wd_sb[:, it, ht * P:(ht + 1) * P]
                    for j, (m0, ln) in enumerate(chunks):
                        nc.tensor.matmul(
                            d_ps[j][:],
                            wslice,
                            acts[:, it, m0:m0 + ln],
                            start=(it == 0),
                            stop=(it == NI - 1),
                        )
                y_sb = [
                    youts.tile([P, ln], f32, tag=f"y{j}", name=f"y{j}_{ht}")
                    for j, (m0, ln) in enumerate(chunks)
                ]
                nc.scalar.copy(y_sb[0][:], d_ps[0][:])
                nc.vector.tensor_copy(y_sb[1][:], d_ps[1][:])
                nc.sync.dma_start(y.ap()[ht, :, 0:c0], y_sb[0][:])
                nc.scalar.dma_start(y.ap()[ht, :, c0:], y_sb[1][:])

    nc.compile()
    return nc


def _pack_w(w_e):
    """[H, I] -> [NI, P, KH*P] bf16 with w[it, p, k*P+i] = w_e[k*P+p, it*P+i]."""
    w4 = w_e.reshape(KH, P, NI, P)
    return np.ascontiguousarray(
        w4.transpose(2, 1, 0, 3).reshape(NI, P, KH * P).astype(BF16)
    )


def kernel(hidden_states, gate_w, wg, wu, wd):
    global LAST_RESULTS
    x = np.ascontiguousarray(np.asarray(hidden_states, dtype=np.float32))
    gw = np.asarray(gate_w, dtype=np.float32)
    wg = np.asarray(wg, dtype=np.float32)
    wu = np.asarray(wu, dtype=np.float32)
    wd = np.asarray(wd, dtype=np.float32)

    # Router on host (part of the dispatch/sharding step).
    logits = np.tanh((x @ gw) / np.float32(SOFTCAP))
    top2 = np.argsort(-logits, axis=1, kind="stable")[:, :TOPK]  # [T, 2]
    v = np.take_along_axis(logits, top2, axis=1)                 # descending
    ex = np.exp(v - v[:, :1])
    pk = (ex / ex.sum(axis=1, keepdims=True)).astype(np.float32)  # [T, 2]

    token_ids, probs_e = [], []
    for e in range(E):
        mask = top2 == e
        rows = np.where(mask.any(axis=1))[0]
        kk = np.argmax(mask[rows], axis=1)
        token_ids.append(rows)
        probs_e.append(pk[rows, kk])

    n_max = max(len(r) for r in token_ids)
    M_PAD = max(64, -(-n_max // 8) * 8)
    if M_PAD % 16:
        M_PAD += 8  # keep both m-chunks equal-sized multiples of 8

    nc = _compiled.get(M_PAD)
    if nc is None:
        nc = _build(M_PAD)
        _compiled[M_PAD] = nc

    in_maps = []
    for e in range(E):
        ids = token_ids[e]
        xe = np.zeros((M_PAD, H), np.float32)
        xe[: len(ids)] = x[ids]
        # [M_PAD, KH, P] -> [KH, P, M_PAD]
        xt_e = np.ascontiguousarray(
            xe.reshape(M_PAD, KH, P).transpose(1, 2, 0).astype(BF16)
        )
        in_maps.append(
            {
                "xt": xt_e,
                "wg": _pack_w(wg[e]),
                "wu": _pack_w(wu[e]),
                "wd": np.ascontiguousarray(wd[e].astype(BF16)),
            }
        )

    res = run_bass_kernel_spmd(nc, in_maps, core_ids=list(range(N_CORES)))
    LAST_RESULTS = res

    out = np.zeros((T, H), np.float32)
    for e in range(E):
        ids = token_ids[e]
        y_e = res.results[e]["y"].reshape(H, M_PAD)  # [ht*P+p, m] = [h, m]
        out[ids] += probs_e[e][:, None] * y_e[:, : len(ids)].T
    return out
